# revision 1
# baseline (speedup 1.0000x reference)
"""EquivariantCrossAttention Trainium2 kernel (8 NeuronCores, SPMD).

kernel(**inputs) takes the FULL unsharded inputs from reference's
setup_inputs() and returns the FULL (B, N, DH) float32 output.

Sharding: flattened query axis (B*N = 4096) split into 8 shards of 512
queries; core i gets queries [512*i, 512*(i+1)) plus its batch's latent
tables. Weights replicated.

Hardcoded problem shapes: B=2 N=2048 L=1024 K=16 CD=2 H=4 DH=128 HD=512.

Algebraic folds done host-side (exact):
  - LayerNorm affines folded into the following Dense weights
  - attention SCALE and eq_w2 folded into wq (W_qm = eq_w2 @ (wq*SCALE))
  - q.k per head via M_h = W_qm_h @ wk_h^T:
      att = g2^T M_h cg + g2.w2v_h + cg.w1v_h + const_h
  - mFFN dense2 and wo merged (W_mo_h = mw2' @ wo_h) and moved after the
    attention sum (softmax weights sum to 1; dense2 affine)
  - mFFN LN normalization folded into attention weights:
      sum_k att*LN(g) = sum_k (att*rstd) g - sum_k att*mean*rstd
  - RFF: t = x@Bs - gathered(p@Bs); sin(2pi t) = Sin(2pi(t - rint t));
    cos(2pi t) = Sin(-2pi|t - rint t| + pi/2)   (ACT Sin domain [-pi,pi])

Device structure per core: 4 query tiles x 128 queries; per tile:
scores via PE -> top-16 via DVE max/max_index/match_replace -> indirect
gather of latent rows -> per-k PE transposes to feature-major -> 4 chunks
of 512 rows (32 queries x 16 neighbors, q-major) through the fused
MLP/attention pipeline. PE matmul row-outputs land on base partitions
{0,32,64,96} and are compacted to [4, *] sbuf tiles with small DMAs.
"""

import sys

sys.path.insert(0, "/opt/trn_rl_repo")

import numpy as np
import ml_dtypes

import concourse.bass as bass
import concourse.bacc as bacc
import concourse.mybir as mybir
import concourse.tile as tile
from concourse.masks import make_identity

F32 = mybir.dt.float32
BF16 = mybir.dt.bfloat16
U32 = mybir.dt.uint32
I32 = mybir.dt.int32
I16 = mybir.dt.int16
AF = mybir.ActivationFunctionType
OP = mybir.AluOpType
AX = mybir.AxisListType

B, N, L, K, CD, H, DH, D = 2, 2048, 1024, 16, 2, 4, 128, 128
HD = H * DH
FQ = 2.0
FV = 2.0
SCALE = 1.0 / float(np.sqrt(DH))
NCORES = 8
NQ = (B * N) // NCORES  # queries per core = 512
QT = NQ // 128  # query tiles per core = 4
NCH = 4  # chunks per query tile
CQ = 128 // NCH  # queries per chunk = 32
CR = CQ * K  # rows per chunk = 512
GELU = AF.Gelu_apprx_tanh
TWO_PI = 2.0 * np.pi

WSPECS = [
    ("rff", [CD, 128], F32),
    ("eq_w1", [128, 128], BF16),
    ("eq_b1", [128, 1], F32),
    ("Mq", [128, 512], BF16),
    ("w1v", [1, 512], BF16),
    ("w2v", [128, 4], BF16),
    ("attconst", [128, 1], F32),
    ("ev_w1", [128, 128], BF16),
    ("ev_b1", [128, 1], F32),
    ("ev_w2", [128, 128], BF16),
    ("ev_b2", [128, 1], F32),
    ("ivw1", [128, 128], BF16),
    ("ivb1", [128, 1], F32),
    ("ivw2g", [128, 512], BF16),
    ("wv", [128, 512], BF16),
    ("WA", [128, 512], BF16),
    ("WB", [128, 512], BF16),
    ("mw1", [128, 128], BF16),
    ("mb1p", [128, 4], F32),
    ("Wmo", [128, 512], BF16),
    ("bmo", [128, 1], F32),
    ("cw1", [128, 128], BF16),
    ("cb1", [128, 1], F32),
    ("cw2g", [128, 128], BF16),
    ("cw2b", [128, 128], BF16),
    ("cb2g1", [128, 1], F32),
    ("cb2b", [128, 1], F32),
]


def _bcast_inner(ap, n):
    """[.., Q] AP -> [.., Q, n] with a stride-0 inner dim (free broadcast)."""
    newap = [list(p) for p in ap.ap] + [[0, n]]
    return bass.AP(ap.tensor, ap.offset, newap)


def build_program():
    nc = bacc.Bacc()

    x_d = nc.declare_dram_parameter("x", [NQ, CD], F32, isOutput=False)
    xh_d = nc.declare_dram_parameter("xh", [NQ, DH], F32, isOutput=False)
    ctbl_d = nc.declare_dram_parameter("ctbl", [L, 3 * D], BF16, isOutput=False)
    ptbl_d = nc.declare_dram_parameter("ptbl", [L, 2 * D], BF16, isOutput=False)
    p2t_d = nc.declare_dram_parameter("p2t", [CD, L], F32, isOutput=False)
    npsq_d = nc.declare_dram_parameter("npsq", [1, L], F32, isOutput=False)
    w_d = {}
    for name, shape, dt in WSPECS:
        w_d[name] = nc.declare_dram_parameter(name, shape, dt, isOutput=False)
    out_d = nc.declare_dram_parameter("out", [NQ, DH], F32, isOutput=True)

    with tile.TileContext(nc) as tc:
        _emit(nc, tc, x_d, xh_d, ctbl_d, ptbl_d, p2t_d, npsq_d, w_d, out_d)
    nc.compile()
    return nc


def _emit(nc, tc, x_d, xh_d, ctbl_d, ptbl_d, p2t_d, npsq_d, w_d, out_d):
    const = tc.alloc_tile_pool(name="const", bufs=1)
    wpool = tc.alloc_tile_pool(name="wpool", bufs=1)
    core = tc.alloc_tile_pool(name="core", bufs=1)
    tl = tc.alloc_tile_pool(name="tl", bufs=1)
    ck = tc.alloc_tile_pool(name="ck", bufs=1)
    psp = tc.alloc_tile_pool(name="psp", bufs=1, space="PSUM")
    _pools = [const, wpool, core, tl, ck, psp]

    _psn = [0]

    def PS(shape, tag, bufs, dtype=F32):
        _psn[0] += 1
        return psp.tile(
            shape, dtype, space="PSUM", tag=tag, bufs=bufs, name=f"ps_{tag}_{_psn[0]}"
        )

    # ---------- constants ----------
    ident = const.tile([128, 128], F32)
    make_identity(nc, ident[:])
    ident_bf = const.tile([128, 128], BF16)
    nc.vector.tensor_copy(out=ident_bf[:], in_=ident[:])

    ones_col_bf = const.tile([128, 1], BF16)
    nc.vector.memset(ones_col_bf[:], 1.0)
    inv128_bf = const.tile([128, 1], BF16)
    nc.vector.memset(inv128_bf[:], 1.0 / 128.0)
    ones_row_bf = const.tile([1, 128], BF16)
    nc.vector.memset(ones_row_bf[:], 1.0)
    ones_row512_bf = const.tile([1, 512], BF16)
    nc.vector.memset(ones_row512_bf[:], 1.0)
    half_row128_bf = const.tile([1, 128], BF16)
    nc.vector.memset(half_row128_bf[:], 0.5)
    onesmat_bf = const.tile([128, 128], BF16)
    nc.vector.memset(onesmat_bf[:], 1.0)
    zeros_row_bf = const.tile([1, 128], BF16)
    nc.vector.memset(zeros_row_bf[:], 0.0)
    zeros_row512_bf = const.tile([1, 512], BF16)
    nc.vector.memset(zeros_row512_bf[:], 0.0)
    ones_row_f32 = const.tile([1, 128], F32)
    nc.vector.memset(ones_row_f32[:], 1.0)
    pihalf = const.tile([128, 1], F32)
    nc.vector.memset(pihalf[:], np.pi / 2.0)
    eps_col = const.tile([128, 1], F32)
    nc.vector.memset(eps_col[:], 1e-6)

    # ---------- weights ----------
    W = {}
    for name, shape, dt in WSPECS:
        wt = wpool.tile(shape, dt, name=f"w_{name}", tag=f"w_{name}")
        nc.sync.dma_start(out=wt[:], in_=w_d[name][:])
        W[name] = wt

    def Wh(name, h, w=128):
        return W[name][:, h * w : (h + 1) * w]

    # ---------- per-core precompute ----------
    x_sb = core.tile([128, QT, CD], F32)
    nc.sync.dma_start(out=x_sb[:], in_=x_d[:].rearrange("(t q) c -> q t c", q=128))
    xsq = core.tile([128, QT], F32)
    xs2 = core.tile([128, QT, CD], F32)
    nc.vector.tensor_tensor(out=xs2[:], in0=x_sb[:], in1=x_sb[:], op=OP.mult)
    nc.vector.tensor_reduce(out=xsq[:], in_=xs2[:], axis=AX.X, op=OP.add)

    x_fm = core.tile([CD, NQ], F32)
    for t in range(QT):
        tp = PS([CD, 128], "tr", 2)
        nc.tensor.transpose(out=tp[:], in_=x_sb[:, t, :], identity=ident[:])
        nc.vector.tensor_copy(out=x_fm[:, 128 * t : 128 * (t + 1)], in_=tp[:])

    p2_fm = core.tile([CD, L], F32)
    nc.sync.dma_start(out=p2_fm[:], in_=p2t_d[:])
    npsq = core.tile([1, L], F32)
    nc.sync.dma_start(out=npsq[:], in_=npsq_d[:])

    # ---------- cFFN on x_h (512 queries at once) ----------
    xh_fm = core.tile([128, NQ], BF16)
    xh_rm = core.tile([128, QT, DH], F32)
    nc.sync.dma_start(out=xh_rm[:], in_=xh_d[:].rearrange("(t q) c -> q t c", q=128))
    for t in range(QT):
        tp = PS([128, 128], "tr", 2)
        nc.tensor.transpose(out=tp[:], in_=xh_rm[:, t, :], identity=ident[:])
        nc.vector.tensor_copy(out=xh_fm[:, 128 * t : 128 * (t + 1)], in_=tp[:])

    c1ps = PS([128, NQ], "mm", 3)
    nc.tensor.matmul(out=c1ps[:], lhsT=W["cw1"][:], rhs=xh_fm[:], start=True, stop=True)
    c1 = core.tile([128, NQ], BF16)
    nc.scalar.activation(out=c1[:], in_=c1ps[:], func=GELU, bias=W["cb1"][:])

    cmean = PS([128, NQ], "rows4", 3)
    nc.tensor.matmul(out=cmean[0:1, :], lhsT=inv128_bf[:], rhs=c1[:], start=True, stop=True)
    c1sq = core.tile([128, NQ], BF16)
    nc.vector.tensor_tensor(out=c1sq[:], in0=c1[:], in1=c1[:], op=OP.mult)
    cmsq = PS([128, NQ], "rows4", 3)
    nc.tensor.matmul(out=cmsq[0:1, :], lhsT=inv128_bf[:], rhs=c1sq[:], start=True, stop=True)

    cm2 = core.tile([1, NQ], F32)
    nc.scalar.square(out=cm2[:], in_=cmean[0:1, :])
    cvar = core.tile([1, NQ], F32)
    nc.vector.tensor_tensor(out=cvar[:], in0=cmsq[0:1, :], in1=cm2[:], op=OP.subtract)
    crstd_bf = core.tile([1, NQ], BF16)
    nc.scalar.activation(
        out=crstd_bf[:], in_=cvar[:], func=AF.Abs_reciprocal_sqrt, bias=eps_col[0:1, :]
    )
    cmr_bf = core.tile([1, NQ], BF16)
    nc.vector.tensor_tensor(out=cmr_bf[:], in0=cmean[0:1, :], in1=crstd_bf[:], op=OP.mult)
    crbc = PS([128, NQ], "mm", 3)
    nc.tensor.matmul(out=crbc[:], lhsT=ones_row_bf[:], rhs=crstd_bf[:], start=True, stop=True)
    cmbc = PS([128, NQ], "mm", 3)
    nc.tensor.matmul(out=cmbc[:], lhsT=ones_row_bf[:], rhs=cmr_bf[:], start=True, stop=True)
    z1c = core.tile([128, NQ], BF16)
    nc.vector.tensor_tensor(out=z1c[:], in0=c1[:], in1=crbc[:], op=OP.mult)
    zc = core.tile([128, NQ], BF16)
    nc.vector.tensor_tensor(out=zc[:], in0=z1c[:], in1=cmbc[:], op=OP.subtract)

    gp1 = core.tile([128, NQ], BF16)
    modadd = core.tile([128, NQ], BF16)
    gps_ = PS([128, NQ], "mm", 3)
    nc.tensor.matmul(out=gps_[:], lhsT=W["cw2g"][:], rhs=zc[:], start=True, stop=True)
    nc.scalar.activation(out=gp1[:], in_=gps_[:], func=AF.Identity, bias=W["cb2g1"][:])
    btps = PS([128, NQ], "mm", 3)
    nc.tensor.matmul(out=btps[:], lhsT=W["cw2b"][:], rhs=zc[:], start=True, stop=True)
    bt = core.tile([128, NQ], BF16)
    nc.scalar.activation(out=bt[:], in_=btps[:], func=AF.Identity, bias=W["cb2b"][:])
    ma1 = core.tile([128, NQ], BF16)
    nc.vector.tensor_scalar(
        out=ma1[:], in0=gp1[:], scalar1=W["ev_b2"][:], scalar2=None, op0=OP.mult
    )
    nc.vector.tensor_tensor(out=modadd[:], in0=ma1[:], in1=bt[:], op=OP.add)

    # ---------- per query tile ----------
    for t in range(QT):
        qs = slice(128 * t, 128 * (t + 1))

        # scores (two 512-wide halves) + top-16
        scores = tl.tile([128, L], F32, tag="scores")
        for s in range(2):
            sl = slice(512 * s, 512 * (s + 1))
            scps = PS([128, 512], "mm", 3)
            nc.tensor.matmul(out=scps[:], lhsT=x_fm[:, qs], rhs=p2_fm[:, sl], start=True, stop=False)
            nc.tensor.matmul(out=scps[:], lhsT=ones_row_f32[:], rhs=npsq[:, sl], start=False, stop=True)
            nc.scalar.copy(out=scores[:, sl], in_=scps[:])
        vals = tl.tile([128, K], F32, tag="vals", bufs=2)
        idxs = tl.tile([128, K], U32, tag="idxs", bufs=2)
        scr2 = tl.tile([128, L], F32, tag="scr2")
        nc.vector.max(out=vals[:, 0:8], in_=scores[:])
        nc.vector.max_index(out=idxs[:, 0:8], in_max=vals[:, 0:8], in_values=scores[:])
        nc.vector.match_replace(
            out=scr2[:], in_to_replace=vals[:, 0:8], in_values=scores[:], imm_value=-1e30
        )
        nc.vector.max(out=vals[:, 8:16], in_=scr2[:])
        nc.vector.max_index(out=idxs[:, 8:16], in_max=vals[:, 8:16], in_values=scr2[:])

        # index prep for dma_gather: idx16[k, q] (int16) replicated across
        # the 8 gpsimd cores' 16-partition blocks
        idxf = tl.tile([128, K], F32, tag="idxf", bufs=2)
        nc.vector.tensor_copy(out=idxf[:], in_=idxs[:])
        idxt_ps = PS([K, 128], "tr", 2)
        nc.tensor.transpose(out=idxt_ps[:], in_=idxf[:], identity=ident[:])
        idx16 = tl.tile([128, 128], I16, tag="idx16", bufs=2)
        nc.vector.tensor_copy(out=idx16[0:16, :], in_=idxt_ps[:])
        for b in range(1, 8):
            nc.sync.dma_start(out=idx16[16 * b : 16 * b + 16, :], in_=idx16[0:16, :])

        # transposed gathers, one per chunk (512 rows each):
        # cfm_c [128, 3, 512]: block 0 = c (feature-major), row0 of block 1/2 =
        # 1/sigma^2 split-bf16; pbt_c [128, 2, 512]: p@Bs split-bf16
        cfm_cs = []
        pbt_cs = []
        for c in range(NCH):
            cfm_c = tl.tile([128, 3, CR], BF16, tag=f"cfm{c}", bufs=2,
                            name=f"cfm{c}_{t}")
            nc.gpsimd.dma_gather(
                out_ap=cfm_c[:], in_ap=ctbl_d[:],
                idxs_ap=idx16[:, 32 * c : 32 * c + 32], num_idxs=CR,
                num_idxs_reg=CR, elem_size=3 * D, transpose=True,
            )
            cfm_cs.append(cfm_c)
            pbt_c = tl.tile([128, 2, CR], BF16, tag=f"pbt{c}", bufs=2,
                            name=f"pbt{c}_{t}")
            nc.gpsimd.dma_gather(
                out_ap=pbt_c[:], in_ap=ptbl_d[:],
                idxs_ap=idx16[:, 32 * c : 32 * c + 32], num_idxs=CR,
                num_idxs_reg=CR, elem_size=2 * D, transpose=True,
            )
            pbt_cs.append(pbt_c)

        # -d^2 row [1, 2048] (q-major) via sbuf->sbuf DMA flatten
        negd2 = tl.tile([128, K], BF16, tag="negd2", bufs=2)
        nc.vector.tensor_scalar(
            out=negd2[:], in0=vals[:], scalar1=xsq[:, t : t + 1], scalar2=None,
            op0=OP.subtract,
        )
        negd2_row = tl.tile([1, 128 * K], BF16, tag="negd2_row", bufs=2)
        nc.sync.dma_start(out=negd2_row[:], in_=negd2[:])

        # t_x = x @ Bs for this tile [128, 128]
        txps = PS([128, 128], "tr", 2)
        nc.tensor.matmul(out=txps[:], lhsT=W["rff"][:], rhs=x_fm[:, qs], start=True, stop=True)
        t_x = tl.tile([128, 128], F32, tag="t_x", bufs=2)
        nc.vector.tensor_copy(out=t_x[:], in_=txps[:])

        zacc = [
            tl.tile([128, 128], F32, tag=f"zacc{h}", name=f"zacc{h}_{t}", bufs=2)
            for h in range(H)
        ]
        den_t = tl.tile([128, 128], F32, tag="den_t", bufs=2, name=f"den_{t}")

        # ---------- chunks ----------
        for c in range(NCH):
            q0 = CQ * c
            qsl = slice(q0, q0 + CQ)
            gq = slice(128 * t + q0, 128 * t + q0 + CQ)
            cg2 = cfm_cs[c][:, 0, :]
            pbt_c = pbt_cs[c]

            # gw chunk [1, 512] = -d^2 * (1/sigma^2)
            inv2 = ck.tile([1, CR], BF16, tag="inv2", bufs=2)
            nc.vector.tensor_tensor(
                out=inv2[:], in0=cfm_cs[c][0:1, 1, :], in1=cfm_cs[c][0:1, 2, :],
                op=OP.add,
            )
            gw_ch = ck.tile([1, CR], BF16, tag="gw_ch", bufs=2)
            nc.vector.tensor_tensor(
                out=gw_ch[:], in0=negd2_row[:, CR * c : CR * (c + 1)], in1=inv2[:],
                op=OP.mult,
            )

            # RFF: t = x@Bs - (p@Bs)  (split-bf16 table)
            tfull0 = ck.tile([128, CQ, K], F32, tag="tfull0", bufs=2)
            nc.vector.tensor_tensor(
                out=tfull0[:],
                in0=_bcast_inner(t_x[:, qsl], K),
                in1=pbt_c[:, 0, :].rearrange("p (a b) -> p a b", a=CQ),
                op=OP.subtract,
            )
            tfull = ck.tile([128, CQ, K], F32, tag="tfull", bufs=2)
            nc.vector.tensor_tensor(
                out=tfull[:], in0=tfull0[:],
                in1=pbt_c[:, 1, :].rearrange("p (a b) -> p a b", a=CQ),
                op=OP.subtract,
            )
            ti = ck.tile([128, CQ, K], I32, tag="ti", bufs=2)
            nc.vector.tensor_copy(out=ti[:], in_=tfull[:])
            fs = ck.tile([128, CQ, K], F32, tag="fs", bufs=2)
            nc.vector.tensor_tensor(out=fs[:], in0=tfull[:], in1=ti[:], op=OP.subtract)
            # cos(2pi t) = sin(2pi((t + 1/4) - rint(t + 1/4))), always in range
            dc0 = ck.tile([128, CQ, K], F32, tag="dc0", bufs=2)
            nc.vector.tensor_scalar(
                out=dc0[:], in0=tfull[:], scalar1=0.25, scalar2=None, op0=OP.add
            )
            ui = ck.tile([128, CQ, K], I32, tag="ui", bufs=2)
            nc.vector.tensor_copy(out=ui[:], in_=dc0[:])
            dc = ck.tile([128, CQ, K], F32, tag="dc", bufs=2)
            nc.vector.tensor_tensor(out=dc[:], in0=dc0[:], in1=ui[:], op=OP.subtract)
            fq = ck.tile([128, CR], BF16, tag="fq", bufs=2)
            fv = ck.tile([128, CR], BF16, tag="fv", bufs=2)
            fs2 = fs[:].rearrange("p a b -> p (a b)")
            dc2 = dc[:].rearrange("p a b -> p (a b)")
            nc.scalar.activation(out=fq[0:64, :], in_=fs2[0:64, :], func=AF.Sin, scale=TWO_PI)
            nc.scalar.activation(
                out=fq[64:128, :], in_=dc2[0:64, :], func=AF.Sin, scale=TWO_PI
            )
            nc.scalar.activation(out=fv[0:64, :], in_=fs2[64:128, :], func=AF.Sin, scale=TWO_PI)
            nc.scalar.activation(
                out=fv[64:128, :], in_=dc2[64:128, :], func=AF.Sin, scale=TWO_PI
            )

            # q path
            g2ps = PS([128, CR], "mm", 3)
            nc.tensor.matmul(out=g2ps[:], lhsT=W["eq_w1"][:], rhs=fq[:], start=True, stop=True)
            g2 = ck.tile([128, CR], BF16, tag="g2", bufs=2)
            nc.scalar.activation(out=g2[:], in_=g2ps[:], func=GELU, bias=W["eq_b1"][:])

            # attention logits -> rows {0,32,64,96} of one psum tile.
            # gw term first (start=True over all 128 rows), heads accumulate.
            attps = PS([128, CR], "rows4", 3)
            nc.tensor.matmul(
                out=attps[:], lhsT=half_row128_bf[:],
                rhs=gw_ch[:], start=True, stop=False,
                skip_group_check=True,
            )
            for h in range(H):
                ups = PS([128, CR], "mm", 3)
                nc.tensor.matmul(out=ups[:], lhsT=Wh("Mq", h), rhs=g2[:], start=True, stop=False)
                nc.tensor.matmul(
                    out=ups[:], lhsT=W["w1v"][:, 128 * h : 128 * (h + 1)],
                    rhs=ones_row512_bf[:], start=False, stop=True,
                )
                qkp = ck.tile([128, CR], BF16, tag="qkp", bufs=2)
                nc.vector.tensor_tensor(out=qkp[:], in0=ups[:], in1=cg2, op=OP.mult)
                nc.tensor.matmul(
                    out=attps[32 * h : 32 * h + 1, :], lhsT=ones_col_bf[:], rhs=qkp[:],
                    start=False, stop=False, tile_position=(0, 32 * h),
                    skip_group_check=True,
                )
                nc.tensor.matmul(
                    out=attps[32 * h : 32 * h + 1, :], lhsT=W["w2v"][:, h : h + 1],
                    rhs=g2[:], start=False, stop=(h == H - 1), tile_position=(0, 32 * h),
                    skip_group_check=True,
                )
            att_e = ck.tile([128, CR], BF16, tag="att_e", bufs=2)
            nc.scalar.activation(out=att_e[:], in_=attps[:], func=AF.Exp, bias=W["attconst"][:])
            nc.vector.tensor_reduce(
                out=den_t[:, qsl], in_=att_e[:].rearrange("p (a b) -> p a b", a=CQ),
                axis=AX.X, op=OP.add,
            )

            # v path
            ev1ps = PS([128, CR], "mm", 3)
            nc.tensor.matmul(out=ev1ps[:], lhsT=W["ev_w1"][:], rhs=fv[:], start=True, stop=True)
            ev1 = ck.tile([128, CR], BF16, tag="ev1", bufs=2)
            nc.scalar.activation(out=ev1[:], in_=ev1ps[:], func=GELU, bias=W["ev_b1"][:])
            ev2ps = PS([128, CR], "mm", 3)
            nc.tensor.matmul(out=ev2ps[:], lhsT=W["ev_w2"][:], rhs=ev1[:], start=True, stop=True)
            mv = ck.tile([128, CQ, K], BF16, tag="mv", bufs=2)
            nc.vector.tensor_tensor(
                out=mv[:], in0=ev2ps[:].rearrange("p (a b) -> p a b", a=CQ),
                in1=_bcast_inner(gp1[:, gq], K), op=OP.mult,
            )
            ivin = ck.tile([128, CQ, K], BF16, tag="ivin", bufs=2)
            nc.vector.tensor_tensor(
                out=ivin[:], in0=mv[:], in1=_bcast_inner(modadd[:, gq], K), op=OP.add
            )
            iv1ps = PS([128, CR], "mm", 3)
            nc.tensor.matmul(
                out=iv1ps[:], lhsT=W["ivw1"][:],
                rhs=ivin[:].rearrange("p a b -> p (a b)"), start=True, stop=True,
            )
            iv1 = ck.tile([128, CR], BF16, tag="iv1", bufs=2)
            nc.scalar.activation(out=iv1[:], in_=iv1ps[:], func=GELU, bias=W["ivb1"][:])

            # iv LN
            ivmean = PS([128, CR], "rows4", 3)
            nc.tensor.matmul(out=ivmean[0:1, :], lhsT=inv128_bf[:], rhs=iv1[:], start=True, stop=True)
            ivsq = ck.tile([128, CR], BF16, tag="ivsq", bufs=2)
            nc.vector.tensor_tensor(out=ivsq[:], in0=iv1[:], in1=iv1[:], op=OP.mult)
            ivmsq = PS([128, CR], "rows4", 3)
            nc.tensor.matmul(out=ivmsq[0:1, :], lhsT=inv128_bf[:], rhs=ivsq[:], start=True, stop=True)
            im2 = ck.tile([1, CR], F32, tag="im2")
            nc.scalar.square(out=im2[:], in_=ivmean[0:1, :])
            ivar = ck.tile([1, CR], F32, tag="ivar")
            nc.vector.tensor_tensor(out=ivar[:], in0=ivmsq[0:1, :], in1=im2[:], op=OP.subtract)
            irstd_bf = ck.tile([1, CR], BF16, tag="irstd_bf")
            nc.scalar.activation(
                out=irstd_bf[:], in_=ivar[:], func=AF.Abs_reciprocal_sqrt,
                bias=eps_col[0:1, :],
            )
            imr_bf = ck.tile([1, CR], BF16, tag="imr_bf")
            nc.vector.tensor_tensor(
                out=imr_bf[:], in0=ivmean[0:1, :], in1=irstd_bf[:], op=OP.mult
            )
            irbc = PS([128, CR], "mm", 3)
            nc.tensor.matmul(out=irbc[:], lhsT=ones_row_bf[:], rhs=irstd_bf[:], start=True, stop=True)
            imbc = PS([128, CR], "mm", 3)
            nc.tensor.matmul(out=imbc[:], lhsT=ones_row_bf[:], rhs=imr_bf[:], start=True, stop=True)
            z1 = ck.tile([128, CR], BF16, tag="z1", bufs=2)
            nc.vector.tensor_tensor(out=z1[:], in0=iv1[:], in1=irbc[:], op=OP.mult)
            ziv = ck.tile([128, CR], BF16, tag="ziv", bufs=2)
            nc.vector.tensor_tensor(out=ziv[:], in0=z1[:], in1=imbc[:], op=OP.subtract)

            # per-head v1 -> gm ; m-LN stats to psum rows
            gms = []
            mmean = PS([128, CR], "rows4", 3)
            msqp = PS([128, CR], "rows4", 3)
            nc.tensor.matmul(
                out=mmean[:], lhsT=zeros_row_bf[:], rhs=zeros_row512_bf[:],
                start=True, stop=False, skip_group_check=True,
            )
            nc.tensor.matmul(
                out=msqp[:], lhsT=zeros_row_bf[:], rhs=zeros_row512_bf[:],
                start=True, stop=False, skip_group_check=True,
            )
            for h in range(H):
                vgps = PS([128, CR], "mm", 3)
                nc.tensor.matmul(out=vgps[:], lhsT=Wh("ivw2g", h), rhs=ziv[:], start=True, stop=True)
                vg_sb = ck.tile([128, CR], BF16, tag="vg_sb", bufs=2)
                nc.vector.tensor_copy(out=vg_sb[:], in_=vgps[:])
                v0ps = PS([128, CR], "mm", 3)
                nc.tensor.matmul(out=v0ps[:], lhsT=Wh("wv", h), rhs=cg2, start=True, stop=True)
                p_sb = ck.tile([128, CR], BF16, tag="p_sb", bufs=2)
                nc.vector.tensor_tensor(out=p_sb[:], in0=v0ps[:], in1=vg_sb[:], op=OP.mult)
                m1ps = PS([128, CR], "mm", 3)
                nc.tensor.matmul(out=m1ps[:], lhsT=W["mw1"][:], rhs=p_sb[:], start=True, stop=False)
                nc.tensor.matmul(out=m1ps[:], lhsT=Wh("WA", h), rhs=cg2, start=False, stop=False)
                nc.tensor.matmul(out=m1ps[:], lhsT=Wh("WB", h), rhs=ziv[:], start=False, stop=True)
                gm = ck.tile([128, CR], BF16, tag=f"gm{h}", bufs=2)
                nc.scalar.activation(out=gm[:], in_=m1ps[:], func=GELU, bias=W["mb1p"][:, h : h + 1])
                gms.append(gm)
                nc.tensor.matmul(
                    out=mmean[32 * h : 32 * h + 1, :], lhsT=inv128_bf[:], rhs=gm[:],
                    start=False, stop=(h == H - 1), tile_position=(0, 32 * h),
                    skip_group_check=True,
                )
                gsq = ck.tile([128, CR], BF16, tag="gsq", bufs=2)
                nc.vector.tensor_tensor(out=gsq[:], in0=gm[:], in1=gm[:], op=OP.mult)
                nc.tensor.matmul(
                    out=msqp[32 * h : 32 * h + 1, :], lhsT=inv128_bf[:], rhs=gsq[:],
                    start=False, stop=(h == H - 1), tile_position=(0, 32 * h),
                    skip_group_check=True,
                )

            mm2 = ck.tile([128, CR], F32, tag="mm2")
            nc.scalar.square(out=mm2[:], in_=mmean[:])
            mvar = ck.tile([128, CR], F32, tag="mvar")
            nc.vector.tensor_tensor(out=mvar[:], in0=msqp[:], in1=mm2[:], op=OP.subtract)
            mrstd = ck.tile([128, CR], BF16, tag="mrstd")
            nc.scalar.activation(
                out=mrstd[:], in_=mvar[:], func=AF.Abs_reciprocal_sqrt, bias=eps_col[:]
            )
            mmr = ck.tile([128, CR], BF16, tag="mmr")
            nc.vector.tensor_tensor(out=mmr[:], in0=mmean[:], in1=mrstd[:], op=OP.mult)

            a2 = ck.tile([128, CR], BF16, tag="a2", bufs=2)
            nc.vector.tensor_tensor(out=a2[:], in0=att_e[:], in1=mrstd[:], op=OP.mult)
            a3 = ck.tile([128, CR], BF16, tag="a3", bufs=2)
            nc.vector.tensor_tensor(out=a3[:], in0=att_e[:], in1=mmr[:], op=OP.mult)
            s3 = ck.tile([128, CQ], F32, tag="s3", bufs=2)
            nc.vector.tensor_reduce(
                out=s3[:], in_=a3[:].rearrange("p (a b) -> p a b", a=CQ), axis=AX.X, op=OP.add
            )
            s3bf = ck.tile([128, CQ], BF16, tag="s3bf", bufs=2)
            nc.vector.tensor_copy(out=s3bf[:], in_=s3[:])

            for h in range(H):
                a2bc = PS([128, CR], "mm", 3)
                nc.tensor.matmul(
                    out=a2bc[:], lhsT=onesmat_bf[32 * h : 32 * h + 1, :],
                    rhs=a2[32 * h : 32 * h + 1, :], start=True, stop=True,
                    tile_position=(32 * h, 0),
                )
                zp = ck.tile([128, CR], BF16, tag="zp", bufs=2)
                nc.vector.tensor_tensor(out=zp[:], in0=gms[h][:], in1=a2bc[:], op=OP.mult)
                nc.vector.tensor_reduce(
                    out=zacc[h][:, qsl], in_=zp[:].rearrange("p (a b) -> p a b", a=CQ),
                    axis=AX.X, op=OP.add,
                )
                s3bc = PS([128, CQ], "tr", 2)
                nc.tensor.matmul(
                    out=s3bc[:], lhsT=onesmat_bf[32 * h : 32 * h + 1, :],
                    rhs=s3bf[32 * h : 32 * h + 1, :], start=True, stop=True,
                    tile_position=(32 * h, 0),
                )
                nc.vector.tensor_tensor(
                    out=zacc[h][:, qsl], in0=zacc[h][:, qsl], in1=s3bc[:], op=OP.subtract
                )

        # softmax denominators: one reciprocal per tile, folded into zacc
        rden_t = tl.tile([128, 128], F32, tag="rden_t", bufs=2, name=f"rden_{t}")
        nc.vector.reciprocal(out=rden_t[:], in_=den_t[:])
        rdbf = tl.tile([128, 128], BF16, tag="rdbf", bufs=2, name=f"rdbf_{t}")
        nc.vector.tensor_copy(out=rdbf[:], in_=rden_t[:])
        for h in range(H):
            rdbc = PS([128, 128], "tr", 2)
            nc.tensor.matmul(
                out=rdbc[:], lhsT=onesmat_bf[32 * h : 32 * h + 1, :],
                rhs=rdbf[32 * h : 32 * h + 1, :], start=True, stop=True,
                tile_position=(32 * h, 0),
            )
            nc.vector.tensor_tensor(out=zacc[h][:], in0=zacc[h][:], in1=rdbc[:], op=OP.mult)

        # tile epilogue
        outps = PS([128, 128], "tr", 2)
        for h in range(H):
            zbf = tl.tile([128, 128], BF16, tag="zbf")
            nc.vector.tensor_copy(out=zbf[:], in_=zacc[h][:])
            nc.tensor.matmul(
                out=outps[:], lhsT=Wh("Wmo", h), rhs=zbf[:], start=(h == 0), stop=(h == H - 1)
            )
        outsb = tl.tile([128, 128], F32, tag="outsb")
        nc.scalar.activation(out=outsb[:], in_=outps[:], func=AF.Identity, bias=W["bmo"][:])
        trp = PS([128, 128], "tr", 2)
        nc.tensor.transpose(out=trp[:], in_=outsb[:], identity=ident[:])
        outrm = tl.tile([128, 128], F32, tag="outrm")
        nc.vector.tensor_copy(out=outrm[:], in_=trp[:])
        nc.sync.dma_start(out=out_d[qs, :], in_=outrm[:])

    for p in reversed(_pools):
        p.release()


# ======================= host side =======================


def _host_prep(inputs):
    f = {k: np.asarray(v, np.float32) for k, v in inputs.items()}

    def bf(x):
        return np.ascontiguousarray(np.asarray(x, np.float32)).astype(ml_dtypes.bfloat16)

    def col(x):
        return np.ascontiguousarray(np.asarray(x, np.float32).reshape(-1, 1))

    rff = np.concatenate([FQ * f["rffq"], FV * f["rffv"]], axis=1)  # [2,128]

    wq_s = f["wq"] * SCALE
    bq_s = f["bq"] * SCALE
    W_qm = f["eq_w2"] @ wq_s
    b_qm = f["eq_b2"] @ wq_s + bq_s
    Mq = np.zeros((128, 512), np.float32)
    w1v = np.zeros((1, 512), np.float32)
    w2v = np.zeros((128, 4), np.float32)
    attconst = np.zeros((128, 1), np.float32)
    for h in range(H):
        sl = slice(128 * h, 128 * (h + 1))
        Wq_h = W_qm[:, sl]
        wk_h = f["wk"][:, sl]
        bk_h = f["bk"][sl]
        bq_h = b_qm[sl]
        Mq[:, sl] = Wq_h @ wk_h.T
        w1v[0, sl] = wk_h @ bq_h
        w2v[:, h] = Wq_h @ bk_h
        attconst[32 * h, 0] = float(bq_h @ bk_h)

    ivw2f = f["ivls"][:, None] * f["ivw2"]
    ivb2f = f["ivb2"] + f["ivlb"] @ f["ivw2"]
    ivw2g = ivw2f[:, :HD]
    ivw2b = ivw2f[:, HD:]
    # bilinear expansion: m1 = mw1.T (v0*vg) + WA.T cg + WB.T ziv + mb1p
    WA = np.zeros((128, 512), np.float32)
    WB = np.zeros((128, 512), np.float32)
    mb1p = np.zeros((128, H), np.float32)
    for h in range(H):
        sl = slice(128 * h, 128 * (h + 1))
        c1_h = 1.0 + ivb2f[:HD][sl]
        bv_h = f["bv"][sl]
        b2_h = ivb2f[HD:][sl]
        WA[:, sl] = f["wv"][:, sl] @ np.diag(c1_h) @ f["mw1"]
        WB[:, sl] = (ivw2g[:, sl] @ np.diag(bv_h) + ivw2b[:, sl]) @ f["mw1"]
        mb1p[:, h] = f["mb1"] + (bv_h * c1_h + b2_h) @ f["mw1"]

    mw2f = f["mls"][:, None] * f["mw2"]
    mb2f = f["mb2"] + f["mlb"] @ f["mw2"]
    Wmo = np.zeros((128, 512), np.float32)
    for h in range(H):
        wo_h = f["wo"][128 * h : 128 * (h + 1), :]
        Wmo[:, 128 * h : 128 * (h + 1)] = mw2f @ wo_h
    bmo = f["bo"] + sum(mb2f @ f["wo"][128 * h : 128 * (h + 1), :] for h in range(H))

    cw2f = f["cls"][:, None] * f["cw2"]
    cb2f = f["cb2"] + f["clb"] @ f["cw2"]

    weights = {
        "rff": np.ascontiguousarray(rff),
        "eq_w1": bf(f["eq_w1"]),
        "eq_b1": col(f["eq_b1"]),
        "Mq": bf(Mq),
        "w1v": bf(w1v),
        "w2v": bf(w2v),
        "attconst": attconst.astype(np.float32),
        "ev_w1": bf(f["ev_w1"]),
        "ev_b1": col(f["ev_b1"]),
        "ev_w2": bf(f["ev_w2"]),
        "ev_b2": col(f["ev_b2"]),
        "ivw1": bf(f["ivw1"]),
        "ivb1": col(f["ivb1"]),
        "ivw2g": bf(ivw2g),
        "wv": bf(f["wv"]),
        "WA": bf(WA),
        "WB": bf(WB),
        "mw1": bf(f["mw1"]),
        "mb1p": np.ascontiguousarray(mb1p),
        "Wmo": bf(Wmo),
        "bmo": col(bmo),
        "cw1": bf(f["cw1"]),
        "cb1": col(f["cb1"]),
        "cw2g": bf(cw2f[:, :DH]),
        "cw2b": bf(cw2f[:, DH:]),
        "cb2g1": col(cb2f[:DH] + 1.0),
        "cb2b": col(cb2f[DH:]),
    }

    x_flat = f["x"].reshape(B * N, CD)
    xh_flat = f["x_h"].reshape(B * N, DH)

    in_maps = []
    for i in range(NCORES):
        b = (i * NQ) // N
        rs = slice(i * NQ, (i + 1) * NQ)
        p_b = f["p"][b]
        c_b = f["c"][b]
        sig_b = f["window_sigma"][b]
        inv2 = 1.0 / (sig_b[:, 0] ** 2)
        i2hi = inv2.astype(ml_dtypes.bfloat16)
        i2lo = (inv2 - i2hi.astype(np.float32)).astype(ml_dtypes.bfloat16)
        ctbl = np.zeros((L, 3 * D), ml_dtypes.bfloat16)
        ctbl[:, :D] = bf(c_b)
        ctbl[:, D] = i2hi
        ctbl[:, 2 * D] = i2lo
        pb = (p_b @ rff).astype(np.float32)
        pbhi = pb.astype(ml_dtypes.bfloat16)
        pblo = (pb - pbhi.astype(np.float32)).astype(ml_dtypes.bfloat16)
        ptbl = np.zeros((L, 2 * D), ml_dtypes.bfloat16)
        ptbl[:, :D] = pbhi
        ptbl[:, D:] = pblo
        m = {
            "x": np.ascontiguousarray(x_flat[rs]),
            "xh": np.ascontiguousarray(xh_flat[rs]),
            "ctbl": ctbl,
            "ptbl": ptbl,
            "p2t": np.ascontiguousarray((2.0 * p_b).T),
            "npsq": np.ascontiguousarray(-(p_b**2).sum(1)[None, :]),
        }
        m.update(weights)
        in_maps.append(m)
    return in_maps


_PROGRAM_CACHE = {}


def kernel(**inputs):
    in_maps = _host_prep(inputs)
    if "nc" not in _PROGRAM_CACHE:
        _PROGRAM_CACHE["nc"] = build_program()
    nc = _PROGRAM_CACHE["nc"]

    from concourse.bass_utils import run_bass_kernel_spmd

    res = run_bass_kernel_spmd(nc, in_maps, core_ids=list(range(NCORES)))
    outs = [np.asarray(res.results[i]["out"], np.float32) for i in range(NCORES)]
    return np.concatenate(outs, axis=0).reshape(B, N, DH)

